# revision 13
# baseline (speedup 1.0000x reference)
"""AdaFusionBlock Trainium2 kernel (8 NeuronCores, data-parallel, no collectives).

Sharding: core = b*4 + q handles batch b, output rows [32q, 32q+32).
Each core receives zero-padded input slabs (x rows +-3, y rows +-12) and
computes its output slab fully locally.

v5: pipeline restructure around the SWDGE gather stream.
  - conv0 folded into the offset-conv weights on the host, so the om conv
    runs directly off [x; y] with K=128 and no conv0 dependency; om runs
    FIRST on PE so the index math / isb pipeline completes early.
  - x0 (residual) computed separately with an M-duplicated w0 so the
    [128]-partition copy needed by the paired convs is free.
  - conv1/conv2 use K=128 tap-pairing: oslab/t1 are stored twice
    (partitions 64-127 hold a one-column-left-shifted copy), so taps
    (ki,0)+(ki,1) run as one K=128 matmul; (ki,2) stays K=64.
  - all PSUM evictions (om bias, x0 bias, z copy r0, deform bias, conv1
    bias+lrelu, conv2 bias) moved to the Scalar engine (activation with
    per-partition bias AP, Lrelu with alpha); DVE keeps only the gather
    combine, folds, residual adds, masks, and index math.
  - gathers: 5 row groups x 9 taps, <=1024 idx per call, queues 0..3,
    grun bufs=6 so 4+ gathers stay in flight.
"""
import sys

sys.path.insert(0, "/opt/trn_rl_repo")

import numpy as np

import concourse.bass as bass
import concourse.bacc as bacc
import concourse.mybir as mybir
from concourse.tile import TileContext
from concourse.masks import make_identity

F32 = mybir.dt.float32
BF16 = mybir.dt.bfloat16
I16 = mybir.dt.int16
I32 = mybir.dt.int32
AOP = mybir.AluOpType
ACTF = mybir.ActivationFunctionType

# geometry
W = 128          # image width
WP = 130         # zero-col-padded width
C = 64           # channels
OH = 32          # output rows per core
EXT = 36         # extended out rows (+-2 halo for the two final convs)
XR = 38          # x-slab rows   [G0-3,  G0+35)
YR = 56          # y-slab rows   [G0-12, G0+44)
NK = 9           # taps
YPX = YR * W     # 7168 slab pixels
N9 = EXT * NK    # 324
NB = N9          # idx stream blocks of 128
OMS = 32         # omt per-row stride (27 used, 32 for transpose blocks)
GRP = [(0, 8), (8, 8), (16, 8), (24, 8), (32, 4)]  # (row base, rows)
C1CH = [(1, 6), (7, 8), (15, 8), (23, 8), (31, 4)]   # conv1 EXT-row chunks
C2CH = [(0, 4), (4, 8), (12, 8), (20, 8), (28, 4)]   # conv2 out-row chunks


def build_nc():
    nc = bacc.Bacc("TRN2", target_bir_lowering=False, num_swdge_queues=4)

    xs = nc.declare_dram_parameter("xs", [C, XR * W], BF16, isOutput=False)
    ys = nc.declare_dram_parameter("ys", [C, YR * W], BF16, isOutput=False)
    w0d = nc.declare_dram_parameter("w0d", [C, 128], BF16, isOutput=False)
    b0d = nc.declare_dram_parameter("b0d", [128, 1], F32, isOutput=False)
    womt = nc.declare_dram_parameter("womt", [NK * 128, 27], BF16, isOutput=False)
    bom = nc.declare_dram_parameter("bom", [27, 1], F32, isOutput=False)
    wdct = nc.declare_dram_parameter("wdct", [NK * C, C], BF16, isOutput=False)
    bdcd = nc.declare_dram_parameter("bdcd", [128, 1], F32, isOutput=False)
    w1p = nc.declare_dram_parameter("w1p", [3 * 128, 128], BF16, isOutput=False)
    w1s = nc.declare_dram_parameter("w1s", [3 * C, 128], BF16, isOutput=False)
    b1d = nc.declare_dram_parameter("b1d", [128, 1], F32, isOutput=False)
    w2p = nc.declare_dram_parameter("w2p", [3 * 128, C], BF16, isOutput=False)
    w2s = nc.declare_dram_parameter("w2s", [3 * C, C], BF16, isOutput=False)
    b2 = nc.declare_dram_parameter("b2", [C, 1], F32, isOutput=False)
    bnd = nc.declare_dram_parameter("bnd", [128, 4], F32, isOutput=False)
    crow = nc.declare_dram_parameter("crow", [128, EXT * NK], F32, isOutput=False)
    cxw = nc.declare_dram_parameter("cxw", [128, NK], F32, isOutput=False)
    m36d = nc.declare_dram_parameter("m36d", [128, EXT], F32, isOutput=False)
    m34d = nc.declare_dram_parameter("m34d", [128, EXT - 2], F32, isOutput=False)
    outp = nc.declare_dram_parameter("out", [C, OH * W], F32, isOutput=True)

    # internal DRAM: k-grouped planes [(0,1),(2,3),(4,5),(6,7),(8,)], blocks
    # hold [kin][rowpair][64] per pixel; x-pair read via elem overlap into b+1
    NBLK = YPX + 256
    BLKG = [256, 256, 256, 256, 128]
    ZOFF = [0, 256 * NBLK, 512 * NBLK, 768 * NBLK, 1024 * NBLK]
    zp = nc.dram_tensor("zp", [1152 * NBLK], BF16)
    idxd = nc.dram_tensor("idxd", [128 * N9], I16)

    from contextlib import ExitStack

    with TileContext(nc) as tc, ExitStack() as es:
        cst = es.enter_context(tc.tile_pool(name="cst", bufs=1))
        big = es.enter_context(tc.tile_pool(name="big", bufs=1))
        ps = es.enter_context(tc.tile_pool(name="ps", bufs=2, space="PSUM"))
        pz = es.enter_context(tc.tile_pool(name="pz", bufs=2, space="PSUM"))
        pt = es.enter_context(tc.tile_pool(name="pt", bufs=2, space="PSUM"))
        gp = es.enter_context(tc.tile_pool(name="gp", bufs=2))
        sm = es.enter_context(tc.tile_pool(name="sm", bufs=1))

        # ---------- loads (inputs on Sync; weights/consts on Scalar HWDGE) ----------
        x0y = big.tile([128, XR * WP], BF16)   # [concat-ch: x | y, XR, WP]
        nc.vector.memset(bass.AP(x0y.tensor, 0, [[XR * WP, 128], [WP, XR], [129, 2]]), 0.0)
        x0y3 = x0y[:, :].rearrange("p (r c) -> p r c", c=WP)
        # lower half <- x slab rows, into cols 1..129
        nc.sync.dma_start(
            out=bass.AP(x0y3.tensor, 1, [[XR * WP, 64], [WP, XR], [1, W]]),
            in_=xs[:, :].rearrange("p (r c) -> p r c", c=W),
        )
        # upper half <- y rows [9, 47) of slab, into cols 1..129
        nc.sync.dma_start(
            out=bass.AP(x0y3.tensor, 64 * (XR * WP) + 1,
                        [[XR * WP, 64], [WP, XR], [1, W]]),
            in_=ys[:, :].rearrange("p (r c) -> p r c", c=W)[:, 9 : 9 + XR, :],
        )
        ysb = big.tile([C, YR * W], BF16)
        nc.sync.dma_start(out=ysb[:, :], in_=ys[:, :])

        w0sb = cst.tile([C, 128], BF16)
        nc.scalar.dma_start(out=w0sb[:, :], in_=w0d[:, :])
        womsb = cst.tile([128, NK * 27], BF16)
        nc.scalar.dma_start(
            out=womsb[:, :].rearrange("p (k o) -> p k o", o=27),
            in_=womt[:, :].rearrange("(k p) o -> p k o", p=128),
        )
        wdcsb = cst.tile([C, 10 * C], BF16)
        nc.vector.memset(wdcsb[:, NK * C :], 0.0)
        nc.scalar.dma_start(
            out=wdcsb[:, : NK * C].rearrange("p (k o) -> p k o", o=C),
            in_=wdct[:, :].rearrange("(k p) o -> p k o", p=C),
        )
        w1psb = cst.tile([128, 3 * 128], BF16)
        nc.scalar.dma_start(
            out=w1psb[:, :].rearrange("p (k o) -> p k o", o=128),
            in_=w1p[:, :].rearrange("(k p) o -> p k o", p=128),
        )
        w1ssb = cst.tile([C, 3 * 128], BF16)
        nc.scalar.dma_start(
            out=w1ssb[:, :].rearrange("p (k o) -> p k o", o=128),
            in_=w1s[:, :].rearrange("(k p) o -> p k o", p=C),
        )
        w2psb = cst.tile([128, 3 * C], BF16)
        nc.scalar.dma_start(
            out=w2psb[:, :].rearrange("p (k o) -> p k o", o=C),
            in_=w2p[:, :].rearrange("(k p) o -> p k o", p=128),
        )
        w2ssb = cst.tile([C, 3 * C], BF16)
        nc.scalar.dma_start(
            out=w2ssb[:, :].rearrange("p (k o) -> p k o", o=C),
            in_=w2s[:, :].rearrange("(k p) o -> p k o", p=C),
        )
        b0sb = cst.tile([128, 1], F32)
        nc.scalar.dma_start(out=b0sb[:, :], in_=b0d[:, :])
        bomsb = cst.tile([27, 1], F32)
        nc.scalar.dma_start(out=bomsb[:, :], in_=bom[:, :])
        bdcsb = cst.tile([128, 1], F32)
        nc.scalar.dma_start(out=bdcsb[:, :], in_=bdcd[:, :])
        b1sb = cst.tile([128, 1], F32)
        nc.scalar.dma_start(out=b1sb[:, :], in_=b1d[:, :])
        b2sb = cst.tile([C, 1], F32)
        nc.scalar.dma_start(out=b2sb[:, :], in_=b2[:, :])
        bndsb = cst.tile([128, 4], F32)
        nc.scalar.dma_start(out=bndsb[:, :], in_=bnd[:, :])
        crowsb = cst.tile([128, EXT * NK], F32)
        nc.scalar.dma_start(out=crowsb[:, :], in_=crow[:, :])
        cxwsb = cst.tile([128, NK], F32)
        nc.scalar.dma_start(out=cxwsb[:, :], in_=cxw[:, :])
        m36sb = cst.tile([128, EXT], F32)
        nc.scalar.dma_start(out=m36sb[:, :], in_=m36d[:, :])
        m34sb = cst.tile([128, EXT - 2], F32)
        nc.scalar.dma_start(out=m34sb[:, :], in_=m34d[:, :])
        ident = cst.tile([128, 128], F32)
        make_identity(nc, ident[:, :])

        # ---------- om conv (9 taps, Cin=128 [x|y], Cout=27) FIRST ----------
        omt = big.tile([128, EXT * OMS], F32)
        omt3 = omt[:, :].rearrange("p (r o) -> p r o", o=OMS)

        def om_chunk(r0q, QRR):
            pm = ps.tile([27, 512], F32, tag="mm", name="pm", space="PSUM")
            for k in range(NK):
                ki, kj = k // 3, k % 3
                nc.tensor.matmul(
                    pm[:, : QRR * W],
                    womsb[:, k * 27 : (k + 1) * 27],
                    bass.AP(x0y3.tensor, (r0q + ki) * WP + kj,
                            [[XR * WP, 128], [WP, QRR], [1, W]]),
                    start=(k == 0), stop=(k == NK - 1),
                )
            omq = gp.tile([27, 8 * W], F32, tag="omq", name="omq")
            omq3 = omq[:, :].rearrange("p (r c) -> p r c", c=W)
            nc.scalar.activation(
                omq3[:, :QRR, :],
                pm[:, : QRR * W].rearrange("p (r c) -> p r c", c=W),
                ACTF.Identity, bias=bomsb[:, :],
            )
            ptr = pt.tile([128, 512], F32, tag="tr", name="ptr", space="PSUM")
            for rq in range(QRR):
                nc.tensor.transpose(
                    ptr[:, rq * 128 : rq * 128 + 27], omq3[:, rq, :],
                    ident[0:27, 0:27],
                )
            nc.scalar.activation(
                bass.AP(omt3.tensor, r0q * OMS, [[EXT * OMS, 128], [OMS, QRR], [1, 27]]),
                ptr[:, :].rearrange("p (r o) -> p r o", o=128)[:, :QRR, 0:27],
                ACTF.Copy,
            )

        for ci in range(9):
            om_chunk(4 * ci, 4)

        # ---------- x0 = W0^T x + b0 (duplicated to both partition halves) ----------
        x0dup = big.tile([128, EXT * W], BF16)
        for ci in range(9):
            px0 = ps.tile([128, 512], F32, tag="mm", name="px0", space="PSUM")
            nc.tensor.matmul(
                px0[:, :],
                w0sb[:, :],
                bass.AP(x0y3.tensor, (1 + 4 * ci) * WP + 1, [[XR * WP, C], [WP, 4], [1, W]]),
                start=True, stop=True,
            )
            nc.scalar.activation(
                x0dup[:, 4 * ci * W : (4 * ci + 4) * W], px0[:, :],
                ACTF.Identity, bias=b0sb[:, :],
            )

        # ---------- offset math (transposed layout [128, EXT, 9]) ----------
        def t9(tag):
            return sm.tile([128, N9], F32, tag=tag, name=tag)

        # offset channels are interleaved: dy_k = om[2k], dx_k = om[2k+1]
        dy = bass.AP(omt.tensor, 0, [[EXT * OMS, 128], [OMS, EXT], [2, NK]])
        dx = bass.AP(omt.tensor, 1, [[EXT * OMS, 128], [OMS, EXT], [2, NK]])
        mr = bass.AP(omt.tensor, 18, [[EXT * OMS, 128], [OMS, EXT], [1, NK]])

        tmp = t9("tmp")
        i32 = sm.tile([128, N9], I32, tag="i32", name="i32")
        dyf = t9("dyf")
        dxf = t9("dxf")
        # floor(x) = ((x - 0.5) + 1.5*2^23) - 1.5*2^23  (fp32 RNE magic round)
        MAGIC = 12582912.0
        nc.vector.tensor_scalar(out=tmp[:, :], in0=dy, scalar1=-0.5, scalar2=MAGIC, op0=AOP.add, op1=AOP.add)
        nc.vector.tensor_scalar(out=dyf[:, :], in0=tmp[:, :], scalar1=-MAGIC, scalar2=None, op0=AOP.add)
        nc.vector.tensor_scalar(out=tmp[:, :], in0=dx, scalar1=-0.5, scalar2=MAGIC, op0=AOP.add, op1=AOP.add)
        nc.vector.tensor_scalar(out=dxf[:, :], in0=tmp[:, :], scalar1=-MAGIC, scalar2=None, op0=AOP.add)

        r0s = t9("r0s")
        nc.vector.tensor_tensor(out=r0s[:, :], in0=crowsb[:, :], in1=dyf[:, :], op=AOP.add)
        x0g = t9("x0g")
        nc.vector.tensor_tensor(
            out=x0g[:, :],
            in0=bass.AP(cxwsb.tensor, 0, [[NK, 128], [0, EXT], [1, NK]]),
            in1=dxf[:, :].rearrange("p (r k) -> p r k", k=NK),
            op=AOP.add,
        )

        # flat index = r0s*128 + x0g  (in-range by construction; clamp for safety)
        nc.vector.tensor_scalar(out=tmp[:, :], in0=r0s[:, :], scalar1=128.0, scalar2=None, op0=AOP.mult)
        nc.vector.tensor_tensor(out=tmp[:, :], in0=tmp[:, :], in1=x0g[:, :], op=AOP.add)
        nc.vector.tensor_scalar(out=tmp[:, :], in0=tmp[:, :], scalar1=0.0, scalar2=6800.0, op0=AOP.max, op1=AOP.min)
        nc.vector.tensor_copy(out=i32[:, :], in_=tmp[:, :])
        idx16 = sm.tile([128, N9], I16, tag="idx16", name="idx16")
        # i32 is (re,k) ordered; stream block b = Bg*9 + k*Rg + r per row-group
        for Bg, Rg in GRP:
            nc.vector.tensor_copy(
                out=bass.AP(idx16.tensor, Bg * 9, [[N9, 128], [Rg, NK], [1, Rg]]),
                in_=bass.AP(i32.tensor, Bg * 9, [[N9, 128], [1, NK], [9, Rg]]),
            )

        # bounce idx to DRAM (addr = lane*NB + b), then reload in the
        # dma_gather wrapped layout (replicated per 16-part group).
        nc.sync.dma_start(
            out=bass.AP(idxd, 0, [[NB, 128], [1, NB]]),
            in_=idx16[:, :],
        )
        isbpre = cst.tile([128, 8 * NB], I16)
        for g in range(8):
            nc.sync.dma_start(
                out=bass.AP(isbpre.tensor, 16 * g * (8 * NB), [[8 * NB, 16], [NB, 8], [1, NB]]),
                in_=bass.AP(idxd, 0, [[NB, 16], [16 * NB, 8], [1, NB]]),
            )
        ty = t9("ty")
        tx = t9("tx")
        nc.vector.tensor_tensor(out=ty[:, :], in0=dy, in1=dyf[:, :], op=AOP.subtract)
        nc.vector.tensor_tensor(out=tx[:, :], in0=dx, in1=dxf[:, :], op=AOP.subtract)
        m2 = t9("m2")
        nc.scalar.activation(m2[:, :], mr, ACTF.Sigmoid)

        va = t9("va")
        vb = t9("vb")
        vv = t9("vv")
        p0t = t9("p0t")
        p1t = t9("p1t")
        q0t = t9("q0t")
        q1t = t9("q1t")

        def valid(src, slo, shi, dst):
            nc.vector.tensor_scalar(out=va[:, :], in0=src[:, :], scalar1=slo, scalar2=None, op0=AOP.is_ge)
            nc.vector.tensor_scalar(out=vb[:, :], in0=src[:, :], scalar1=shi, scalar2=None, op0=AOP.is_le)
            nc.vector.tensor_tensor(out=dst[:, :], in0=va[:, :], in1=vb[:, :], op=AOP.mult)

        # p0t = 2*(1-ty)*m2*vy0 ; p1t = 2*ty*m2*vy1
        valid(r0s, bndsb[:, 0:1], bndsb[:, 1:2], vv)
        nc.vector.tensor_scalar(out=p0t[:, :], in0=ty[:, :], scalar1=-2.0, scalar2=2.0, op0=AOP.mult, op1=AOP.add)
        nc.vector.tensor_tensor(out=p0t[:, :], in0=p0t[:, :], in1=m2[:, :], op=AOP.mult)
        nc.vector.tensor_tensor(out=p0t[:, :], in0=p0t[:, :], in1=vv[:, :], op=AOP.mult)
        valid(r0s, bndsb[:, 2:3], bndsb[:, 3:4], vv)
        nc.vector.tensor_scalar(out=p1t[:, :], in0=ty[:, :], scalar1=2.0, scalar2=None, op0=AOP.mult)
        nc.vector.tensor_tensor(out=p1t[:, :], in0=p1t[:, :], in1=m2[:, :], op=AOP.mult)
        nc.vector.tensor_tensor(out=p1t[:, :], in0=p1t[:, :], in1=vv[:, :], op=AOP.mult)
        # q0t = (1-tx)*vx0 ; q1t = tx*vx1
        valid(x0g, 0.0, 127.0, vv)
        nc.vector.tensor_scalar(out=q0t[:, :], in0=tx[:, :], scalar1=-1.0, scalar2=1.0, op0=AOP.mult, op1=AOP.add)
        nc.vector.tensor_tensor(out=q0t[:, :], in0=q0t[:, :], in1=vv[:, :], op=AOP.mult)
        valid(x0g, -1.0, 126.0, vv)
        nc.vector.tensor_tensor(out=q1t[:, :], in0=tx[:, :], in1=vv[:, :], op=AOP.mult)

        # paired-corner weight tiles, interleaved per (re,k):
        # u01[p, (re*9+k)*2 + j] = weight of corner (rj, x0); u23 same for x1
        u01 = sm.tile([128, 2 * N9], BF16, tag="u01", name="u01")
        u23 = sm.tile([128, 2 * N9], BF16, tag="u23", name="u23")
        nc.vector.tensor_tensor(
            out=bass.AP(u01.tensor, 0, [[2 * N9, 128], [2, N9]]),
            in0=p0t[:, :], in1=q0t[:, :], op=AOP.mult)
        nc.vector.tensor_tensor(
            out=bass.AP(u01.tensor, 1, [[2 * N9, 128], [2, N9]]),
            in0=p1t[:, :], in1=q0t[:, :], op=AOP.mult)
        nc.vector.tensor_tensor(
            out=bass.AP(u23.tensor, 0, [[2 * N9, 128], [2, N9]]),
            in0=p0t[:, :], in1=q1t[:, :], op=AOP.mult)
        nc.vector.tensor_tensor(
            out=bass.AP(u23.tensor, 1, [[2 * N9, 128], [2, N9]]),
            in0=p1t[:, :], in1=q1t[:, :], op=AOP.mult)

        isb = cst.tile([128, NB * 8], I16)
        nc.vector.tensor_copy(
            out=bass.AP(isb.tensor, 0, [[8 * NB, 128], [8, NB], [1, 8]]),
            in_=bass.AP(isbpre.tensor, 0, [[8 * NB, 128], [1, NB], [NB, 8]]),
        )

        # ---------- Z planes (pair-interleaved bf16, 6-slot ring, batch-3 writes) ----------
        # slot layout per partition: [g<4: kin(2) x r(2) x 64 = 256e] x4, [g4: r(2) x 64]
        ZD = 6       # ring depth
        ZB = 3       # rows per zp write batch
        zpr = big.tile([128, ZD * 2432], BF16)
        ZFS = ZD * 2432  # zpr free size (partition stride)

        def zp_write_batch(r0, nrow, s0, eng):
            # rows [r0, r0+nrow) from zpr slots [s0, s0+nrow); one DMA per k-group
            for g in range(4):
                eng.dma_start(
                    out=bass.AP(zp, ZOFF[g] + r0 * W * 256,
                                [[256, 128], [128 * 256, nrow], [1, 256]]),
                    in_=bass.AP(zpr.tensor, s0 * 2432 + g * 512,
                                [[ZFS, 128], [2432, nrow], [1, 256]]),
                )
            eng.dma_start(
                out=bass.AP(zp, ZOFF[4] + r0 * W * 128, [[128, 128], [128 * 128, nrow], [1, 128]]),
                in_=bass.AP(zpr.tensor, s0 * 2432 + 2048, [[ZFS, 128], [2432, nrow], [1, 128]]),
            )

        zstate = {"nbatch": 0}

        def z_block(lo, hi):
            for ch in range(lo, hi):
                pzt = pz.tile([128, 10 * C], F32, tag="pz", name="pzt", space="PSUM")
                nc.tensor.matmul(
                    pzt[:, 0 : 8 * C],
                    ysb[:, ch * W : (ch + 1) * W],
                    wdcsb[:, 0 : 8 * C],
                    start=True, stop=True,
                )
                nc.tensor.matmul(
                    pzt[:, 8 * C : 10 * C],
                    ysb[:, ch * W : (ch + 1) * W],
                    wdcsb[:, 8 * C : 10 * C],
                    start=True, stop=True,
                )
                sl = ch % ZD
                # slot r0 (this row): all 10 slots in one strided copy (Scalar)
                nc.scalar.activation(
                    bass.AP(zpr.tensor, sl * 2432, [[ZFS, 128], [512, 5], [128, 2], [1, C]]),
                    pzt[:, :].rearrange("p (g j o) -> p g j o", j=2, o=C),
                    ACTF.Copy,
                )
                # slot r1 into previous row's slot (offset +64) (Vector)
                if ch > 0:
                    nc.vector.tensor_copy(
                        out=bass.AP(zpr.tensor, (ch - 1) % ZD * 2432 + C, [[ZFS, 128], [512, 5], [128, 2], [1, C]]),
                        in_=pzt[:, :].rearrange("p (g j o) -> p g j o", j=2, o=C),
                    )
                    # batch-write fully completed rows [ch-ZB .. ch-1] when aligned
                    if ch % ZB == 0 and ch >= ZB:
                        eng = nc.scalar if zstate["nbatch"] % 2 == 0 else nc.sync
                        zp_write_batch(ch - ZB, ZB, (ch - ZB) % ZD, eng)
                        zstate["nbatch"] += 1

        z_block(0, YR)
        zp_write_batch(54, 2, 54 % ZD, nc.scalar)  # row 55 r1 garbage, never read

        # ---------- gather + combine + per-group tail ----------
        oslab = big.tile([128, EXT * WP], BF16)
        os3 = oslab[:, :].rearrange("p (r c) -> p r c", c=WP)
        nc.vector.memset(oslab[:, :], 0.0)
        T1S = (EXT - 2) * WP
        t1 = big.tile([128, T1S], BF16)
        t13 = t1[:, :].rearrange("p (r c) -> p r c", c=WP)
        nc.vector.memset(t1[:, :], 0.0)

        for gi, (Bg, Rg) in enumerate(GRP):
            CNT = (Bg + Rg + 18) * 128  # zp blocks addressable by this group
            sacc01 = gp.tile([128, Rg * 128], BF16, tag="sacc01", name=f"s01_{gi}", bufs=2)
            sacc23 = gp.tile([128, Rg * 128], BF16, tag="sacc23", name=f"s23_{gi}", bufs=2)
            for k in range(NK):
                g4, kin = k // 2, k % 2
                blk = BLKG[g4]
                esz = blk + 128
                grun = gp.tile([128, Rg * 384], BF16, tag="grun", name="grun", bufs=5)
                gv = grun[:, : Rg * esz].rearrange("p (r e) -> p r e", e=esz)
                base = Bg * 9 + k * Rg  # stream block offset
                nc.gpsimd.dma_gather(
                    gv[:, :, :],
                    bass.AP(zp, ZOFF[g4] + kin * 128, [[blk, CNT], [1, esz]]),
                    isb[:, base * 8 : (base + Rg) * 8],
                    num_idxs=Rg * 128,
                    num_idxs_reg=Rg * 128,
                    elem_size=esz,
                    elem_step=blk,
                    queue_num=(gi * NK + k) % 4,
                )
                # paired-corner combine: (r0,r1)x64ch contiguous at x0 / x1
                ub = (Bg * 9 + k) * 2
                uap01 = bass.AP(u01.tensor, ub, [[2 * N9, 128], [18, Rg], [1, 2], [0, C]])
                uap23 = bass.AP(u23.tensor, ub, [[2 * N9, 128], [18, Rg], [1, 2], [0, C]])
                gva = bass.AP(grun.tensor, 0, [[Rg * 384, 128], [esz, Rg], [C, 2], [1, C]])
                gvb = bass.AP(grun.tensor, blk, [[Rg * 384, 128], [esz, Rg], [C, 2], [1, C]])
                if k == 0:
                    s01v = bass.AP(sacc01.tensor, 0, [[Rg * 128, 128], [128, Rg], [C, 2], [1, C]])
                    s23v = bass.AP(sacc23.tensor, 0, [[Rg * 128, 128], [128, Rg], [C, 2], [1, C]])
                    nc.vector.tensor_tensor(out=s01v, in0=gva, in1=uap01, op=AOP.mult)
                    nc.vector.tensor_tensor(out=s23v, in0=gvb, in1=uap23, op=AOP.mult)
                else:
                    tmpc = gp.tile([128, Rg * 128], BF16, tag="tmpc", name="tmpc", bufs=2)
                    tcv = bass.AP(tmpc.tensor, 0, [[Rg * 128, 128], [128, Rg], [C, 2], [1, C]])
                    nc.vector.tensor_tensor(out=tcv, in0=gva, in1=uap01, op=AOP.mult)
                    nc.vector.tensor_tensor(out=sacc01[:, :], in0=sacc01[:, :], in1=tmpc[:, :], op=AOP.add)
                    tmpd = gp.tile([128, Rg * 128], BF16, tag="tmpd", name="tmpd", bufs=2)
                    tdv = bass.AP(tmpd.tensor, 0, [[Rg * 128, 128], [128, Rg], [C, 2], [1, C]])
                    nc.vector.tensor_tensor(out=tdv, in0=gvb, in1=uap23, op=AOP.mult)
                    nc.vector.tensor_tensor(out=sacc23[:, :], in0=sacc23[:, :], in1=tmpd[:, :], op=AOP.add)
            # fold: accq = sacc01 + sacc23 (f32), then fold the (r0,r1) pair,
            # duplicating the 64 output channels into both halves of the free
            # dim so the transpose lands them on both partition halves.
            accq = gp.tile([128, Rg * 128], F32, tag="accq", name=f"accq{gi}", bufs=2)
            nc.vector.tensor_tensor(out=accq[:, :], in0=sacc01[:, :], in1=sacc23[:, :], op=AOP.add)
            acc = gp.tile([128, Rg * 128], F32, tag="acc", name=f"acc{gi}", bufs=2)
            nc.vector.tensor_tensor(
                out=acc[:, :].rearrange("p (r d) -> p r d", d=128),
                in0=bass.AP(accq.tensor, 0, [[Rg * 128, 128], [128, Rg], [0, 2], [1, C]]),
                in1=bass.AP(accq.tensor, C, [[Rg * 128, 128], [128, Rg], [0, 2], [1, C]]),
                op=AOP.add,
            )
            acc3 = acc[:, :].rearrange("p (r d) -> p r d", d=128)

            # transpose back (batches of 4 rows) + bdc + x0 residual -> oslab
            # (partitions 64-127 get the copy shifted one column left)
            for rb in range(Rg // 4):
                ptb = pt.tile([128, 512], F32, tag="tr", name="ptb", space="PSUM")
                for i in range(4):
                    nc.tensor.transpose(
                        ptb[:, i * W : (i + 1) * W],
                        acc3[:, rb * 4 + i, :],
                        ident[:, :],
                    )
                tdc = gp.tile([128, 512], BF16, tag="tdc", bufs=2)
                nc.scalar.activation(tdc[:, :], ptb[:, :], ACTF.Identity, bias=bdcsb[:, :])
                re0 = Bg + rb * 4
                nc.vector.tensor_tensor(
                    out=bass.AP(os3.tensor, re0 * WP + 1, [[EXT * WP, C], [WP, 4], [1, W]]),
                    in0=tdc[0:C, :].rearrange("p (r c) -> p r c", c=W),
                    in1=bass.AP(x0dup.tensor, re0 * W, [[EXT * W, C], [W, 4], [1, W]]),
                    op=AOP.add,
                )
                nc.vector.tensor_tensor(
                    out=bass.AP(os3.tensor, 64 * (EXT * WP) + re0 * WP, [[EXT * WP, 64], [WP, 4], [1, W]]),
                    in0=tdc[64:128, :].rearrange("p (r c) -> p r c", c=W),
                    in1=bass.AP(x0dup.tensor, 64 * (EXT * W) + re0 * W, [[EXT * W, 64], [W, 4], [1, W]]),
                    op=AOP.add,
                )
            # zero out-of-image rows for this group (both halves)
            nc.vector.tensor_tensor(
                out=os3[:, Bg : Bg + Rg, :],
                in0=os3[:, Bg : Bg + Rg, :],
                in1=bass.AP(m36sb.tensor, Bg, [[EXT, 128], [1, Rg], [0, WP]]),
                op=AOP.mult,
            )

            # conv1 chunk enabled by this group (+ lrelu via Scalar engine)
            r0c0, nrow_g = C1CH[gi]
            nsub = 0
            while nsub < nrow_g:
                nr_h = min(4, nrow_g - nsub)
                r0c = r0c0 + nsub
                pc1 = ps.tile([128, 512], F32, tag="mm", name="pc1", space="PSUM")[:, : nr_h * W]
                for ki in range(3):
                    nc.tensor.matmul(
                        pc1[:, : nr_h * W],
                        w1psb[:, ki * 128 : (ki + 1) * 128],
                        bass.AP(os3.tensor, (r0c - 1 + ki) * WP, [[EXT * WP, 128], [WP, nr_h], [1, W]]),
                        start=(ki == 0), stop=False,
                    )
                    nc.tensor.matmul(
                        pc1[:, : nr_h * W],
                        w1ssb[:, ki * 128 : (ki + 1) * 128],
                        bass.AP(os3.tensor, (r0c - 1 + ki) * WP + 2, [[EXT * WP, C], [WP, nr_h], [1, W]]),
                        start=False, stop=(ki == 2),
                    )
                # t1 row (T1 coords = EXT row - 1): bias on ACT, lrelu on DVE
                nc.scalar.activation(
                    bass.AP(t13.tensor, (r0c - 1) * WP + 1, [[T1S, C], [WP, nr_h], [1, W]]),
                    pc1[0:C, : nr_h * W].rearrange("p (r c) -> p r c", c=W),
                    ACTF.Identity, bias=b1sb[0:C, :],
                )
                nc.scalar.activation(
                    bass.AP(t13.tensor, 64 * T1S + (r0c - 1) * WP, [[T1S, 64], [WP, nr_h], [1, W]]),
                    pc1[64:128, : nr_h * W].rearrange("p (r c) -> p r c", c=W),
                    ACTF.Identity, bias=b1sb[64:128, :],
                )
                nc.vector.scalar_tensor_tensor(
                    out=t13[:, r0c - 1 : r0c - 1 + nr_h, :],
                    in0=t13[:, r0c - 1 : r0c - 1 + nr_h, :],
                    scalar=0.2,
                    in1=t13[:, r0c - 1 : r0c - 1 + nr_h, :],
                    op0=AOP.mult,
                    op1=AOP.max,
                )
                nc.vector.tensor_tensor(
                    out=t13[:, r0c - 1 : r0c - 1 + nr_h, :],
                    in0=t13[:, r0c - 1 : r0c - 1 + nr_h, :],
                    in1=bass.AP(m34sb.tensor, r0c - 1, [[EXT - 2, 128], [1, nr_h], [0, WP]]),
                    op=AOP.mult,
                )
                nsub += nr_h

            # conv2 chunk + residual + store
            o00, nrow_g2 = C2CH[gi]
            nsub = 0
            while nsub < nrow_g2:
                nr_h = min(4, nrow_g2 - nsub)
                o0 = o00 + nsub
                pc2 = ps.tile([C, 512], F32, tag="mm", name="pc2", space="PSUM")[:, : nr_h * W]
                for ki in range(3):
                    # conv2 out row o reads t1 rows (o+ki) in T1 coords
                    nc.tensor.matmul(
                        pc2[:, : nr_h * W],
                        w2psb[:, ki * C : (ki + 1) * C],
                        bass.AP(t13.tensor, (o0 + ki) * WP, [[T1S, 128], [WP, nr_h], [1, W]]),
                        start=(ki == 0), stop=False,
                    )
                    nc.tensor.matmul(
                        pc2[:, : nr_h * W],
                        w2ssb[:, ki * C : (ki + 1) * C],
                        bass.AP(t13.tensor, (o0 + ki) * WP + 2, [[T1S, C], [WP, nr_h], [1, W]]),
                        start=False, stop=(ki == 2),
                    )
                tf = gp.tile([C, 512], F32, tag="tf", name="tf", bufs=2)
                nc.scalar.activation(tf[:, : nr_h * W], pc2[:, :], ACTF.Identity, bias=b2sb[:, :])
                nc.vector.tensor_tensor(
                    out=tf[:, : nr_h * W].rearrange("p (r c) -> p r c", c=W),
                    in0=tf[:, : nr_h * W].rearrange("p (r c) -> p r c", c=W),
                    in1=bass.AP(os3.tensor, (o0 + 2) * WP + 1, [[EXT * WP, C], [WP, nr_h], [1, W]]),
                    op=AOP.add,
                )
                nc.sync.dma_start(
                    out=outp[:, o0 * W : (o0 + nr_h) * W], in_=tf[:, : nr_h * W]
                )
                nsub += nr_h

    nc.finalize()
    return nc


# ---------------- host side ----------------

_NC_CACHE = None


def _get_nc():
    global _NC_CACHE
    if _NC_CACHE is None:
        _NC_CACHE = build_nc()
    return _NC_CACHE


def _prep_core(inputs, b, q):
    G0 = 32 * q
    x = inputs["x"][b]  # [64, 128, 128]
    y = inputs["y"][b]

    def slab(img, lo, rows):
        out = np.zeros((C, rows, W), np.float32)
        for i in range(rows):
            g = lo + i
            if 0 <= g < 128:
                out[:, i, :] = img[:, g, :]
        return out

    import ml_dtypes
    bf = ml_dtypes.bfloat16
    xsl = slab(x, G0 - 3, XR).reshape(C, XR * W).astype(bf)
    ysl = slab(y, G0 - 12, YR).reshape(C, YR * W).astype(bf)

    w0 = inputs["w0"][:, :, 0, 0]                       # [o', e]
    w0d = np.concatenate([w0.T, w0.T], axis=1).copy().astype(bf)  # [e, 2*o]
    b0d = np.tile(inputs["b0"].reshape(C, 1), (2, 1)).astype(np.float32)

    # fold conv0 into the om conv's x half: om_x = (w_om_x @ w0) * x
    w_om = inputs["w_om"]                               # [27, 128, 3, 3]
    w_omx = np.einsum("ocij,ce->oeij", w_om[:, :C], w0).astype(np.float32)
    w_omc = np.concatenate([w_omx, w_om[:, C:]], axis=1)  # [27, 128, 3, 3]
    womt = (np.transpose(w_omc, (2, 3, 1, 0)).reshape(NK * 128, 27).copy()).astype(bf)

    wdct = (np.transpose(inputs["w_dc"], (2, 3, 1, 0)).reshape(NK * C, C).copy()).astype(bf)
    bdcd = np.tile(inputs["b_dc"].reshape(C, 1), (2, 1)).astype(np.float32)

    # conv1: M = 128 (dup); conv2: M = 64
    w1 = inputs["w1"]
    w1p = np.zeros((3, 128, 128), np.float32)
    w1s = np.zeros((3, C, 128), np.float32)
    for ki in range(3):
        lt0 = w1[:, :, ki, 0].T      # [c, o]
        lt1 = w1[:, :, ki, 1].T
        lt2 = w1[:, :, ki, 2].T
        w1p[ki, :C, :C] = lt0
        w1p[ki, :C, C:] = lt0
        w1p[ki, C:, :C] = lt1
        w1p[ki, C:, C:] = lt1
        w1s[ki, :, :C] = lt2
        w1s[ki, :, C:] = lt2
    b1dv = np.tile(inputs["b1"].reshape(C, 1), (2, 1)).astype(np.float32)

    w2 = inputs["w2"]
    w2p = np.zeros((3, 128, C), np.float32)
    w2s = np.zeros((3, C, C), np.float32)
    for ki in range(3):
        w2p[ki, :C, :] = w2[:, :, ki, 0].T
        w2p[ki, C:, :] = w2[:, :, ki, 1].T
        w2s[ki, :, :] = w2[:, :, ki, 2].T

    lo = 12.0 - G0
    hi = 139.0 - G0
    bnd = np.tile(np.array([[lo, hi, lo - 1.0, hi - 1.0]], np.float32), (128, 1))

    re_idx = np.arange(EXT)[:, None]
    ki = (np.arange(NK) // 3)[None, :]
    kj = (np.arange(NK) % 3)[None, :]
    crow_row = (re_idx + ki + 9).astype(np.float32).reshape(1, EXT * NK)
    crow = np.tile(crow_row, (128, 1))
    wv = np.arange(128)[:, None].astype(np.float32)
    cxw = (wv - 1.0 + kj.astype(np.float32))  # [128, 9]

    def rowmask(lo_r, rows):
        g = lo_r + np.arange(rows)
        m = ((g >= 0) & (g < 128)).astype(np.float32)
        return np.tile(m[None, :], (128, 1))

    return {
        "xs": xsl,
        "ys": ysl,
        "w0d": w0d,
        "b0d": b0d,
        "womt": womt,
        "bom": inputs["b_om"].reshape(27, 1).astype(np.float32),
        "wdct": wdct,
        "bdcd": bdcd,
        "w1p": w1p.reshape(3 * 128, 128).astype(bf),
        "w1s": w1s.reshape(3 * C, 128).astype(bf),
        "b1d": b1dv,
        "w2p": w2p.reshape(3 * 128, C).astype(bf),
        "w2s": w2s.reshape(3 * C, C).astype(bf),
        "b2": inputs["b2"].reshape(C, 1).astype(np.float32),
        "bnd": bnd,
        "crow": crow,
        "cxw": cxw.astype(np.float32),
        "m36d": rowmask(G0 - 2, EXT),
        "m34d": rowmask(G0 - 1, EXT - 2),
    }


def make_in_maps(inputs):
    inputs = {k: np.asarray(v, np.float32) for k, v in inputs.items()}
    return [_prep_core(inputs, core // 4, core % 4) for core in range(8)]


def kernel(**inputs):
    from concourse.bass_utils import run_bass_kernel_spmd

    nc = _get_nc()
    in_maps = make_in_maps(inputs)
    res = run_bass_kernel_spmd(nc, in_maps, core_ids=list(range(8)))
    out = np.zeros((2, C, 128, W), np.float32)
    for core in range(8):
        b, q = core // 4, core % 4
        out[b, :, 32 * q : 32 * q + 32, :] = res.results[core]["out"].reshape(C, OH, W)
    return out


# revision 21
# speedup vs baseline: 1.0982x; 1.0982x over previous
"""AdaFusionBlock Trainium2 kernel (8 NeuronCores, data-parallel, no collectives).

Sharding: core = b*4 + q handles batch b, output rows [32q, 32q+32).
Each core receives zero-padded input slabs (x rows +-3, y rows +-12) and
computes its output slab fully locally.

v5: pipeline restructure around the SWDGE gather stream.
  - conv0 folded into the offset-conv weights on the host, so the om conv
    runs directly off [x; y] with K=128 and no conv0 dependency; om runs
    FIRST on PE so the index math / isb pipeline completes early.
  - x0 (residual) computed separately with an M-duplicated w0 so the
    [128]-partition copy needed by the paired convs is free.
  - conv1/conv2 use K=128 tap-pairing: oslab/t1 are stored twice
    (partitions 64-127 hold a one-column-left-shifted copy), so taps
    (ki,0)+(ki,1) run as one K=128 matmul; (ki,2) stays K=64.
  - all PSUM evictions (om bias, x0 bias, z copy r0, deform bias, conv1
    bias+lrelu, conv2 bias) moved to the Scalar engine (activation with
    per-partition bias AP, Lrelu with alpha); DVE keeps only the gather
    combine, folds, residual adds, masks, and index math.
  - gathers: 5 row groups x 9 taps, <=1024 idx per call, queues 0..3,
    grun bufs=6 so 4+ gathers stay in flight.
"""
import sys

sys.path.insert(0, "/opt/trn_rl_repo")

import numpy as np

import concourse.bass as bass
import concourse.bacc as bacc
import concourse.mybir as mybir
from concourse.tile import TileContext
from concourse.masks import make_identity

F32 = mybir.dt.float32
BF16 = mybir.dt.bfloat16
I16 = mybir.dt.int16
I32 = mybir.dt.int32
AOP = mybir.AluOpType
ACTF = mybir.ActivationFunctionType

# geometry
W = 128          # image width
WP = 130         # zero-col-padded width
C = 64           # channels
OH = 32          # output rows per core
EXT = 36         # extended out rows (+-2 halo for the two final convs)
XR = 38          # x-slab rows   [G0-3,  G0+35)
YR = 56          # y-slab rows   [G0-12, G0+44)
NK = 9           # taps
YPX = YR * W     # 7168 slab pixels
N9 = EXT * NK    # 324
NB = N9          # idx stream blocks of 128
OMS = 32         # omt per-row stride (27 used, 32 for transpose blocks)
GRP = [(0, 8), (8, 8), (16, 8), (24, 8), (32, 4)]  # (row base, rows)
C1CH = [(1, 6), (7, 8), (15, 8), (23, 8), (31, 4)]   # conv1 EXT-row chunks
C2CH = [(0, 4), (4, 8), (12, 8), (20, 8), (28, 4)]   # conv2 out-row chunks


def build_nc():
    nc = bacc.Bacc("TRN2", target_bir_lowering=False, num_swdge_queues=4)

    xs = nc.declare_dram_parameter("xs", [C, XR * W], BF16, isOutput=False)
    ys = nc.declare_dram_parameter("ys", [C, YR * W], BF16, isOutput=False)
    w0d = nc.declare_dram_parameter("w0d", [C, 128], BF16, isOutput=False)
    b0d = nc.declare_dram_parameter("b0d", [128, 1], F32, isOutput=False)
    womt = nc.declare_dram_parameter("womt", [NK * 128, 27], BF16, isOutput=False)
    bom = nc.declare_dram_parameter("bom", [27, 1], F32, isOutput=False)
    wdct = nc.declare_dram_parameter("wdct", [NK * C, C], BF16, isOutput=False)
    bdcd = nc.declare_dram_parameter("bdcd", [128, 1], F32, isOutput=False)
    w1p = nc.declare_dram_parameter("w1p", [3 * 128, 128], BF16, isOutput=False)
    w1s = nc.declare_dram_parameter("w1s", [3 * C, 128], BF16, isOutput=False)
    b1d = nc.declare_dram_parameter("b1d", [128, 1], F32, isOutput=False)
    w2p = nc.declare_dram_parameter("w2p", [3 * 128, C], BF16, isOutput=False)
    w2s = nc.declare_dram_parameter("w2s", [3 * C, C], BF16, isOutput=False)
    b2 = nc.declare_dram_parameter("b2", [C, 1], F32, isOutput=False)
    bnd = nc.declare_dram_parameter("bnd", [128, 4], F32, isOutput=False)
    crow = nc.declare_dram_parameter("crow", [128, EXT * NK], F32, isOutput=False)
    cxw = nc.declare_dram_parameter("cxw", [128, NK], F32, isOutput=False)
    m36d = nc.declare_dram_parameter("m36d", [128, EXT], F32, isOutput=False)
    m34d = nc.declare_dram_parameter("m34d", [128, EXT - 2], F32, isOutput=False)
    outp = nc.declare_dram_parameter("out", [C, OH * W], F32, isOutput=True)

    # internal DRAM: k-grouped planes [(0,1),(2,3),(4,5),(6,7),(8,)], blocks
    # hold [kin][rowpair][64] per pixel; x-pair read via elem overlap into b+1
    NBLK = YPX + 256
    BLKG = [256, 256, 256, 256, 128]
    ZOFF = [0, 256 * NBLK, 512 * NBLK, 768 * NBLK, 1024 * NBLK]
    zp = nc.dram_tensor("zp", [1152 * NBLK], BF16)
    idxd = nc.dram_tensor("idxd", [128 * N9], I16)

    from contextlib import ExitStack

    with TileContext(nc) as tc, ExitStack() as es:
        cst = es.enter_context(tc.tile_pool(name="cst", bufs=1))
        big = es.enter_context(tc.tile_pool(name="big", bufs=1))
        ps = es.enter_context(tc.tile_pool(name="ps", bufs=2, space="PSUM"))
        pz = es.enter_context(tc.tile_pool(name="pz", bufs=2, space="PSUM"))
        pt = es.enter_context(tc.tile_pool(name="pt", bufs=2, space="PSUM"))
        gp = es.enter_context(tc.tile_pool(name="gp", bufs=2))
        sm = es.enter_context(tc.tile_pool(name="sm", bufs=1))

        # ---------- loads (inputs on Sync; weights/consts on Scalar HWDGE) ----------
        x0y = big.tile([128, XR * WP], BF16)   # [concat-ch: x | y, XR, WP]
        nc.vector.memset(bass.AP(x0y.tensor, 0, [[XR * WP, 128], [WP, XR], [129, 2]]), 0.0)
        x0y3 = x0y[:, :].rearrange("p (r c) -> p r c", c=WP)
        # lower half <- x slab rows, into cols 1..129
        nc.sync.dma_start(
            out=bass.AP(x0y3.tensor, 1, [[XR * WP, 64], [WP, XR], [1, W]]),
            in_=xs[:, :].rearrange("p (r c) -> p r c", c=W),
        )
        # upper half <- y rows [9, 47) of slab, into cols 1..129
        nc.sync.dma_start(
            out=bass.AP(x0y3.tensor, 64 * (XR * WP) + 1,
                        [[XR * WP, 64], [WP, XR], [1, W]]),
            in_=ys[:, :].rearrange("p (r c) -> p r c", c=W)[:, 9 : 9 + XR, :],
        )
        ysb = big.tile([C, YR * W], BF16)
        nc.sync.dma_start(out=ysb[:, :], in_=ys[:, :])

        w0sb = cst.tile([C, 128], BF16)
        nc.scalar.dma_start(out=w0sb[:, :], in_=w0d[:, :])
        womsb = cst.tile([128, NK * 27], BF16)
        nc.scalar.dma_start(
            out=womsb[:, :].rearrange("p (k o) -> p k o", o=27),
            in_=womt[:, :].rearrange("(k p) o -> p k o", p=128),
        )
        wdcsb = cst.tile([C, 10 * C], BF16)
        nc.vector.memset(wdcsb[:, NK * C :], 0.0)
        nc.scalar.dma_start(
            out=wdcsb[:, : NK * C].rearrange("p (k o) -> p k o", o=C),
            in_=wdct[:, :].rearrange("(k p) o -> p k o", p=C),
        )
        w1psb = cst.tile([128, 3 * 128], BF16)
        nc.scalar.dma_start(
            out=w1psb[:, :].rearrange("p (k o) -> p k o", o=128),
            in_=w1p[:, :].rearrange("(k p) o -> p k o", p=128),
        )
        w1ssb = cst.tile([C, 3 * 128], BF16)
        nc.scalar.dma_start(
            out=w1ssb[:, :].rearrange("p (k o) -> p k o", o=128),
            in_=w1s[:, :].rearrange("(k p) o -> p k o", p=C),
        )
        w2psb = cst.tile([128, 3 * C], BF16)
        nc.scalar.dma_start(
            out=w2psb[:, :].rearrange("p (k o) -> p k o", o=C),
            in_=w2p[:, :].rearrange("(k p) o -> p k o", p=128),
        )
        w2ssb = cst.tile([C, 3 * C], BF16)
        nc.scalar.dma_start(
            out=w2ssb[:, :].rearrange("p (k o) -> p k o", o=C),
            in_=w2s[:, :].rearrange("(k p) o -> p k o", p=C),
        )
        b0sb = cst.tile([128, 1], F32)
        nc.scalar.dma_start(out=b0sb[:, :], in_=b0d[:, :])
        bomsb = cst.tile([27, 1], F32)
        nc.scalar.dma_start(out=bomsb[:, :], in_=bom[:, :])
        bdcsb = cst.tile([128, 1], F32)
        nc.scalar.dma_start(out=bdcsb[:, :], in_=bdcd[:, :])
        b1sb = cst.tile([128, 1], F32)
        nc.scalar.dma_start(out=b1sb[:, :], in_=b1d[:, :])
        b2sb = cst.tile([C, 1], F32)
        nc.scalar.dma_start(out=b2sb[:, :], in_=b2[:, :])
        bndsb = cst.tile([128, 4], F32)
        nc.scalar.dma_start(out=bndsb[:, :], in_=bnd[:, :])
        crowsb = cst.tile([128, EXT * NK], F32)
        nc.scalar.dma_start(out=crowsb[:, :], in_=crow[:, :])
        cxwsb = cst.tile([128, NK], F32)
        nc.scalar.dma_start(out=cxwsb[:, :], in_=cxw[:, :])
        m36sb = cst.tile([128, EXT], F32)
        nc.scalar.dma_start(out=m36sb[:, :], in_=m36d[:, :])
        m34sb = cst.tile([128, EXT - 2], F32)
        nc.scalar.dma_start(out=m34sb[:, :], in_=m34d[:, :])
        ident = cst.tile([128, 128], F32)
        make_identity(nc, ident[:, :])

        # ---------- om conv (9 taps, Cin=128 [x|y], Cout=27) FIRST ----------
        omt = big.tile([128, EXT * OMS], F32)
        omt3 = omt[:, :].rearrange("p (r o) -> p r o", o=OMS)

        omq_pend = []

        def om_chunk(r0q, QRR):
            pm = ps.tile([27, 512], F32, tag="mm", name="pm", space="PSUM")
            for k in range(NK):
                ki, kj = k // 3, k % 3
                nc.tensor.matmul(
                    pm[:, : QRR * W],
                    womsb[:, k * 27 : (k + 1) * 27],
                    bass.AP(x0y3.tensor, (r0q + ki) * WP + kj,
                            [[XR * WP, 128], [WP, QRR], [1, W]]),
                    start=(k == 0), stop=(k == NK - 1),
                )
            omq = gp.tile([27, 8 * W], F32, tag="omq", name="omq", bufs=3)
            omq3 = omq[:, :].rearrange("p (r c) -> p r c", c=W)
            nc.scalar.activation(
                omq3[:, :QRR, :],
                pm[:, : QRR * W].rearrange("p (r c) -> p r c", c=W),
                ACTF.Identity, bias=bomsb[:, :],
            )
            omq_pend.append((r0q, QRR, omq3))

        def om_flush_tr():
            # transposes for the oldest pending chunk (pipelined one chunk
            # behind the matmuls so PE never stalls on the Scalar evict)
            r0q, QRR, omq3 = omq_pend.pop(0)
            ptr = pt.tile([128, 512], F32, tag="tr", name="ptr", space="PSUM")
            for rq in range(QRR):
                nc.tensor.transpose(
                    ptr[:, rq * 128 : rq * 128 + 27], omq3[:, rq, :],
                    ident[0:27, 0:27],
                )
            nc.scalar.activation(
                bass.AP(omt3.tensor, r0q * OMS, [[EXT * OMS, 128], [OMS, QRR], [1, 27]]),
                ptr[:, :].rearrange("p (r o) -> p r o", o=128)[:, :QRR, 0:27],
                ACTF.Copy,
            )


        # ---------- Z planes (pair-interleaved bf16, 6-slot ring, batch-3 writes) ----------
        # slot layout per partition: [g<4: kin(2) x r(2) x 64 = 256e] x4, [g4: r(2) x 64]
        ZD = 6       # ring depth
        ZB = 3       # rows per zp write batch
        zpr = big.tile([128, ZD * 2432], BF16)
        ZFS = ZD * 2432  # zpr free size (partition stride)

        def zp_write_batch(r0, nrow, s0, eng):
            # rows [r0, r0+nrow) from zpr slots [s0, s0+nrow); one DMA per
            # k-group, split across both HWDGE queues to halve issue pacing
            eng2 = nc.sync if eng is nc.scalar else nc.scalar
            for g in range(4):
                e = eng if g % 2 == 0 else eng2
                e.dma_start(
                    out=bass.AP(zp, ZOFF[g] + r0 * W * 256,
                                [[256, 128], [128 * 256, nrow], [1, 256]]),
                    in_=bass.AP(zpr.tensor, s0 * 2432 + g * 512,
                                [[ZFS, 128], [2432, nrow], [1, 256]]),
                )
            eng.dma_start(
                out=bass.AP(zp, ZOFF[4] + r0 * W * 128, [[128, 128], [128 * 128, nrow], [1, 128]]),
                in_=bass.AP(zpr.tensor, s0 * 2432 + 2048, [[ZFS, 128], [2432, nrow], [1, 128]]),
            )

        zstate = {"nbatch": 0}

        def z_block(lo, hi):
            for ch in range(lo, hi):
                pzt = pz.tile([128, 10 * C], F32, tag="pz", name="pzt", space="PSUM")
                nc.tensor.matmul(
                    pzt[:, 0 : 8 * C],
                    ysb[:, ch * W : (ch + 1) * W],
                    wdcsb[:, 0 : 8 * C],
                    start=True, stop=True,
                )
                nc.tensor.matmul(
                    pzt[:, 8 * C : 10 * C],
                    ysb[:, ch * W : (ch + 1) * W],
                    wdcsb[:, 8 * C : 10 * C],
                    start=True, stop=True,
                )
                sl = ch % ZD
                # slot r0 (this row): all 10 slots in one strided copy (Scalar)
                nc.scalar.activation(
                    bass.AP(zpr.tensor, sl * 2432, [[ZFS, 128], [512, 5], [128, 2], [1, C]]),
                    pzt[:, :].rearrange("p (g j o) -> p g j o", j=2, o=C),
                    ACTF.Copy,
                )
                # slot r1 into previous row's slot (offset +64) (Vector)
                if ch > 0:
                    nc.vector.tensor_copy(
                        out=bass.AP(zpr.tensor, (ch - 1) % ZD * 2432 + C, [[ZFS, 128], [512, 5], [128, 2], [1, C]]),
                        in_=pzt[:, :].rearrange("p (g j o) -> p g j o", j=2, o=C),
                    )
                    # batch-write fully completed rows [ch-ZB .. ch-1] when aligned
                    if ch % ZB == 0 and ch >= ZB:
                        eng = nc.scalar if zstate["nbatch"] % 2 == 0 else nc.sync
                        zp_write_batch(ch - ZB, ZB, (ch - ZB) % ZD, eng)
                        zstate["nbatch"] += 1

        # interleave om chunks with z row blocks on PE; om transposes ride one
        # chunk behind.  z rows 0..54 only (row 55 blocks are never gathered).
        ZBL = ((0, 11), (11, 22), (22, 33), (33, 44))
        for ci in range(4):
            om_chunk(4 * (2 * ci), 4)
            om_flush_tr()
            om_chunk(4 * (2 * ci + 1), 4)
            om_flush_tr()
            z_block(*ZBL[ci])
        om_chunk(32, 4)
        om_flush_tr()

        # ---------- offset math (transposed layout [128, EXT, 9]) ----------
        def t9(tag):
            return sm.tile([128, N9], F32, tag=tag, name=tag)

        # offset channels are interleaved: dy_k = om[2k], dx_k = om[2k+1]
        dy = bass.AP(omt.tensor, 0, [[EXT * OMS, 128], [OMS, EXT], [2, NK]])
        dx = bass.AP(omt.tensor, 1, [[EXT * OMS, 128], [OMS, EXT], [2, NK]])
        mr = bass.AP(omt.tensor, 18, [[EXT * OMS, 128], [OMS, EXT], [1, NK]])

        tmp = t9("tmp")
        i32 = sm.tile([128, N9], I32, tag="i32", name="i32")
        dyf = t9("dyf")
        dxf = t9("dxf")
        # floor(x) = ((x - 0.5) + 1.5*2^23) - 1.5*2^23  (fp32 RNE magic round)
        MAGIC = 12582912.0
        nc.vector.tensor_scalar(out=tmp[:, :], in0=dy, scalar1=-0.5, scalar2=MAGIC, op0=AOP.add, op1=AOP.add)
        nc.vector.tensor_scalar(out=dyf[:, :], in0=tmp[:, :], scalar1=-MAGIC, scalar2=None, op0=AOP.add)
        nc.vector.tensor_scalar(out=tmp[:, :], in0=dx, scalar1=-0.5, scalar2=MAGIC, op0=AOP.add, op1=AOP.add)
        nc.vector.tensor_scalar(out=dxf[:, :], in0=tmp[:, :], scalar1=-MAGIC, scalar2=None, op0=AOP.add)

        r0s = t9("r0s")
        nc.vector.tensor_tensor(out=r0s[:, :], in0=crowsb[:, :], in1=dyf[:, :], op=AOP.add)
        x0g = t9("x0g")
        nc.vector.tensor_tensor(
            out=x0g[:, :],
            in0=bass.AP(cxwsb.tensor, 0, [[NK, 128], [0, EXT], [1, NK]]),
            in1=dxf[:, :].rearrange("p (r k) -> p r k", k=NK),
            op=AOP.add,
        )

        # flat index = r0s*128 + x0g  (in-range by construction; clamp for safety)
        nc.vector.tensor_scalar(out=tmp[:, :], in0=r0s[:, :], scalar1=128.0, scalar2=None, op0=AOP.mult)
        nc.vector.tensor_tensor(out=tmp[:, :], in0=tmp[:, :], in1=x0g[:, :], op=AOP.add)
        nc.vector.tensor_scalar(out=tmp[:, :], in0=tmp[:, :], scalar1=0.0, scalar2=6800.0, op0=AOP.max, op1=AOP.min)
        nc.vector.tensor_copy(out=i32[:, :], in_=tmp[:, :])
        idx16 = sm.tile([128, N9], I16, tag="idx16", name="idx16")
        # i32 is (re,k) ordered; stream block b = Bg*9 + k*Rg + r per row-group
        for Bg, Rg in GRP:
            nc.vector.tensor_copy(
                out=bass.AP(idx16.tensor, Bg * 9, [[N9, 128], [Rg, NK], [1, Rg]]),
                in_=bass.AP(i32.tensor, Bg * 9, [[N9, 128], [1, NK], [9, Rg]]),
            )

        # bounce idx to DRAM (addr = lane*NB + b), then reload in the
        # dma_gather wrapped layout (replicated per 16-part group).
        nc.gpsimd.dma_start(
            out=bass.AP(idxd, 0, [[NB, 128], [1, NB]]),
            in_=idx16[:, :],
        )
        isbpre = cst.tile([128, 8 * NB], I16)
        for g in range(8):
            nc.gpsimd.dma_start(
                out=bass.AP(isbpre.tensor, 16 * g * (8 * NB), [[8 * NB, 16], [NB, 8], [1, NB]]),
                in_=bass.AP(idxd, 0, [[NB, 16], [16 * NB, 8], [1, NB]]),
            )
        ty = t9("ty")
        tx = t9("tx")
        nc.vector.tensor_tensor(out=ty[:, :], in0=dy, in1=dyf[:, :], op=AOP.subtract)
        nc.vector.tensor_tensor(out=tx[:, :], in0=dx, in1=dxf[:, :], op=AOP.subtract)
        m2 = t9("m2")
        nc.scalar.activation(m2[:, :], mr, ACTF.Sigmoid)

        va = t9("va")
        vb = t9("vb")
        vv = t9("vv")
        p0t = t9("p0t")
        p1t = t9("p1t")
        q0t = t9("q0t")
        q1t = t9("q1t")

        def valid(src, slo, shi, dst):
            nc.vector.tensor_scalar(out=va[:, :], in0=src[:, :], scalar1=slo, scalar2=None, op0=AOP.is_ge)
            nc.vector.tensor_scalar(out=vb[:, :], in0=src[:, :], scalar1=shi, scalar2=None, op0=AOP.is_le)
            nc.vector.tensor_tensor(out=dst[:, :], in0=va[:, :], in1=vb[:, :], op=AOP.mult)

        # p0t = 2*(1-ty)*m2*vy0 ; p1t = 2*ty*m2*vy1
        valid(r0s, bndsb[:, 0:1], bndsb[:, 1:2], vv)
        nc.vector.tensor_scalar(out=p0t[:, :], in0=ty[:, :], scalar1=-2.0, scalar2=2.0, op0=AOP.mult, op1=AOP.add)
        nc.vector.tensor_tensor(out=p0t[:, :], in0=p0t[:, :], in1=m2[:, :], op=AOP.mult)
        nc.vector.tensor_tensor(out=p0t[:, :], in0=p0t[:, :], in1=vv[:, :], op=AOP.mult)
        valid(r0s, bndsb[:, 2:3], bndsb[:, 3:4], vv)
        nc.vector.tensor_scalar(out=p1t[:, :], in0=ty[:, :], scalar1=2.0, scalar2=None, op0=AOP.mult)
        nc.vector.tensor_tensor(out=p1t[:, :], in0=p1t[:, :], in1=m2[:, :], op=AOP.mult)
        nc.vector.tensor_tensor(out=p1t[:, :], in0=p1t[:, :], in1=vv[:, :], op=AOP.mult)
        # q0t = (1-tx)*vx0 ; q1t = tx*vx1
        valid(x0g, 0.0, 127.0, vv)
        nc.vector.tensor_scalar(out=q0t[:, :], in0=tx[:, :], scalar1=-1.0, scalar2=1.0, op0=AOP.mult, op1=AOP.add)
        nc.vector.tensor_tensor(out=q0t[:, :], in0=q0t[:, :], in1=vv[:, :], op=AOP.mult)
        valid(x0g, -1.0, 126.0, vv)
        nc.vector.tensor_tensor(out=q1t[:, :], in0=tx[:, :], in1=vv[:, :], op=AOP.mult)

        # paired-corner weight tiles, interleaved per (re,k):
        # u01[p, (re*9+k)*2 + j] = weight of corner (rj, x0); u23 same for x1
        u01 = sm.tile([128, 2 * N9], BF16, tag="u01", name="u01")
        u23 = sm.tile([128, 2 * N9], BF16, tag="u23", name="u23")
        nc.vector.tensor_tensor(
            out=bass.AP(u01.tensor, 0, [[2 * N9, 128], [2, N9]]),
            in0=p0t[:, :], in1=q0t[:, :], op=AOP.mult)
        nc.vector.tensor_tensor(
            out=bass.AP(u01.tensor, 1, [[2 * N9, 128], [2, N9]]),
            in0=p1t[:, :], in1=q0t[:, :], op=AOP.mult)
        nc.vector.tensor_tensor(
            out=bass.AP(u23.tensor, 0, [[2 * N9, 128], [2, N9]]),
            in0=p0t[:, :], in1=q1t[:, :], op=AOP.mult)
        nc.vector.tensor_tensor(
            out=bass.AP(u23.tensor, 1, [[2 * N9, 128], [2, N9]]),
            in0=p1t[:, :], in1=q1t[:, :], op=AOP.mult)

        isb = cst.tile([128, NB * 8], I16)
        nc.vector.tensor_copy(
            out=bass.AP(isb.tensor, 0, [[8 * NB, 128], [8, NB], [1, 8]]),
            in_=bass.AP(isbpre.tensor, 0, [[8 * NB, 128], [1, NB], [NB, 8]]),
        )

        z_block(44, 55)

        # x0 = W0^T x + b0 (duplicated to both partition halves)
        x0dup = big.tile([128, EXT * W], BF16)
        for ci in range(9):
            px0 = ps.tile([128, 512], F32, tag="mm", name="px0", space="PSUM")
            nc.tensor.matmul(
                px0[:, :],
                w0sb[:, :],
                bass.AP(x0y3.tensor, (1 + 4 * ci) * WP + 1, [[XR * WP, C], [WP, 4], [1, W]]),
                start=True, stop=True,
            )
            nc.scalar.activation(
                x0dup[:, 4 * ci * W : (4 * ci + 4) * W], px0[:, :],
                ACTF.Identity, bias=b0sb[:, :],
            )

        # ---------- gather + combine + per-group tail ----------
        oslab = big.tile([128, EXT * WP], BF16)
        os3 = oslab[:, :].rearrange("p (r c) -> p r c", c=WP)
        nc.vector.memset(oslab[:, :], 0.0)
        T1S = (EXT - 2) * WP
        t1 = big.tile([128, T1S], BF16)
        t13 = t1[:, :].rearrange("p (r c) -> p r c", c=WP)
        nc.vector.memset(t1[:, :], 0.0)

        for gi, (Bg, Rg) in enumerate(GRP):
            CNT = (Bg + Rg + 18) * 128  # zp blocks addressable by this group
            sacc01 = gp.tile([128, Rg * 128], BF16, tag="sacc01", name=f"s01_{gi}", bufs=2)
            sacc23 = gp.tile([128, Rg * 128], BF16, tag="sacc23", name=f"s23_{gi}", bufs=2)
            for k in range(NK):
                g4, kin = k // 2, k % 2
                blk = BLKG[g4]
                esz = blk + 128
                grun = gp.tile([128, Rg * 384], BF16, tag="grun", name="grun", bufs=5)
                gv = grun[:, : Rg * esz].rearrange("p (r e) -> p r e", e=esz)
                base = Bg * 9 + k * Rg  # stream block offset
                nc.gpsimd.dma_gather(
                    gv[:, :, :],
                    bass.AP(zp, ZOFF[g4] + kin * 128, [[blk, CNT], [1, esz]]),
                    isb[:, base * 8 : (base + Rg) * 8],
                    num_idxs=Rg * 128,
                    num_idxs_reg=Rg * 128,
                    elem_size=esz,
                    elem_step=blk,
                    queue_num=(gi * NK + k) % 4,
                )
                # paired-corner combine: (r0,r1)x64ch contiguous at x0 / x1
                ub = (Bg * 9 + k) * 2
                uap01 = bass.AP(u01.tensor, ub, [[2 * N9, 128], [18, Rg], [1, 2], [0, C]])
                uap23 = bass.AP(u23.tensor, ub, [[2 * N9, 128], [18, Rg], [1, 2], [0, C]])
                gva = bass.AP(grun.tensor, 0, [[Rg * 384, 128], [esz, Rg], [C, 2], [1, C]])
                gvb = bass.AP(grun.tensor, blk, [[Rg * 384, 128], [esz, Rg], [C, 2], [1, C]])
                if k == 0:
                    s01v = bass.AP(sacc01.tensor, 0, [[Rg * 128, 128], [128, Rg], [C, 2], [1, C]])
                    s23v = bass.AP(sacc23.tensor, 0, [[Rg * 128, 128], [128, Rg], [C, 2], [1, C]])
                    nc.vector.tensor_tensor(out=s01v, in0=gva, in1=uap01, op=AOP.mult)
                    nc.vector.tensor_tensor(out=s23v, in0=gvb, in1=uap23, op=AOP.mult)
                else:
                    tmpc = gp.tile([128, Rg * 128], BF16, tag="tmpc", name="tmpc", bufs=2)
                    tcv = bass.AP(tmpc.tensor, 0, [[Rg * 128, 128], [128, Rg], [C, 2], [1, C]])
                    nc.vector.tensor_tensor(out=tcv, in0=gva, in1=uap01, op=AOP.mult)
                    nc.vector.tensor_tensor(out=sacc01[:, :], in0=sacc01[:, :], in1=tmpc[:, :], op=AOP.add)
                    tmpd = gp.tile([128, Rg * 128], BF16, tag="tmpd", name="tmpd", bufs=2)
                    tdv = bass.AP(tmpd.tensor, 0, [[Rg * 128, 128], [128, Rg], [C, 2], [1, C]])
                    nc.vector.tensor_tensor(out=tdv, in0=gvb, in1=uap23, op=AOP.mult)
                    nc.vector.tensor_tensor(out=sacc23[:, :], in0=sacc23[:, :], in1=tmpd[:, :], op=AOP.add)
            # fold: accq = sacc01 + sacc23 (f32), then fold the (r0,r1) pair,
            # duplicating the 64 output channels into both halves of the free
            # dim so the transpose lands them on both partition halves.
            accq = gp.tile([128, Rg * 128], F32, tag="accq", name=f"accq{gi}", bufs=2)
            nc.vector.tensor_tensor(out=accq[:, :], in0=sacc01[:, :], in1=sacc23[:, :], op=AOP.add)
            acc = gp.tile([128, Rg * 128], F32, tag="acc", name=f"acc{gi}", bufs=2)
            nc.vector.tensor_tensor(
                out=acc[:, :].rearrange("p (r d) -> p r d", d=128),
                in0=bass.AP(accq.tensor, 0, [[Rg * 128, 128], [128, Rg], [0, 2], [1, C]]),
                in1=bass.AP(accq.tensor, C, [[Rg * 128, 128], [128, Rg], [0, 2], [1, C]]),
                op=AOP.add,
            )
            acc3 = acc[:, :].rearrange("p (r d) -> p r d", d=128)

            # transpose back (batches of 4 rows) + bdc + x0 residual -> oslab
            # (partitions 64-127 get the copy shifted one column left)
            for rb in range(Rg // 4):
                ptb = pt.tile([128, 512], F32, tag="tr", name="ptb", space="PSUM")
                for i in range(4):
                    nc.tensor.transpose(
                        ptb[:, i * W : (i + 1) * W],
                        acc3[:, rb * 4 + i, :],
                        ident[:, :],
                    )
                tdc = gp.tile([128, 512], BF16, tag="tdc", bufs=2)
                nc.scalar.activation(tdc[:, :], ptb[:, :], ACTF.Identity, bias=bdcsb[:, :])
                re0 = Bg + rb * 4
                nc.vector.tensor_tensor(
                    out=bass.AP(os3.tensor, re0 * WP + 1, [[EXT * WP, C], [WP, 4], [1, W]]),
                    in0=tdc[0:C, :].rearrange("p (r c) -> p r c", c=W),
                    in1=bass.AP(x0dup.tensor, re0 * W, [[EXT * W, C], [W, 4], [1, W]]),
                    op=AOP.add,
                )
                nc.vector.tensor_tensor(
                    out=bass.AP(os3.tensor, 64 * (EXT * WP) + re0 * WP, [[EXT * WP, 64], [WP, 4], [1, W]]),
                    in0=tdc[64:128, :].rearrange("p (r c) -> p r c", c=W),
                    in1=bass.AP(x0dup.tensor, 64 * (EXT * W) + re0 * W, [[EXT * W, 64], [W, 4], [1, W]]),
                    op=AOP.add,
                )
            # zero out-of-image rows for this group (both halves)
            nc.vector.tensor_tensor(
                out=os3[:, Bg : Bg + Rg, :],
                in0=os3[:, Bg : Bg + Rg, :],
                in1=bass.AP(m36sb.tensor, Bg, [[EXT, 128], [1, Rg], [0, WP]]),
                op=AOP.mult,
            )

            # conv1 chunk enabled by this group (+ lrelu via Scalar engine)
            r0c0, nrow_g = C1CH[gi]
            nsub = 0
            while nsub < nrow_g:
                nr_h = min(4, nrow_g - nsub)
                r0c = r0c0 + nsub
                pc1 = ps.tile([128, 512], F32, tag="mm", name="pc1", space="PSUM")[:, : nr_h * W]
                for ki in range(3):
                    nc.tensor.matmul(
                        pc1[:, : nr_h * W],
                        w1psb[:, ki * 128 : (ki + 1) * 128],
                        bass.AP(os3.tensor, (r0c - 1 + ki) * WP, [[EXT * WP, 128], [WP, nr_h], [1, W]]),
                        start=(ki == 0), stop=False,
                    )
                    nc.tensor.matmul(
                        pc1[:, : nr_h * W],
                        w1ssb[:, ki * 128 : (ki + 1) * 128],
                        bass.AP(os3.tensor, (r0c - 1 + ki) * WP + 2, [[EXT * WP, C], [WP, nr_h], [1, W]]),
                        start=False, stop=(ki == 2),
                    )
                # t1 row (T1 coords = EXT row - 1): bias on ACT, lrelu on DVE
                nc.scalar.activation(
                    bass.AP(t13.tensor, (r0c - 1) * WP + 1, [[T1S, C], [WP, nr_h], [1, W]]),
                    pc1[0:C, : nr_h * W].rearrange("p (r c) -> p r c", c=W),
                    ACTF.Identity, bias=b1sb[0:C, :],
                )
                nc.scalar.activation(
                    bass.AP(t13.tensor, 64 * T1S + (r0c - 1) * WP, [[T1S, 64], [WP, nr_h], [1, W]]),
                    pc1[64:128, : nr_h * W].rearrange("p (r c) -> p r c", c=W),
                    ACTF.Identity, bias=b1sb[64:128, :],
                )
                nc.vector.scalar_tensor_tensor(
                    out=t13[:, r0c - 1 : r0c - 1 + nr_h, :],
                    in0=t13[:, r0c - 1 : r0c - 1 + nr_h, :],
                    scalar=0.2,
                    in1=t13[:, r0c - 1 : r0c - 1 + nr_h, :],
                    op0=AOP.mult,
                    op1=AOP.max,
                )
                nc.vector.tensor_tensor(
                    out=t13[:, r0c - 1 : r0c - 1 + nr_h, :],
                    in0=t13[:, r0c - 1 : r0c - 1 + nr_h, :],
                    in1=bass.AP(m34sb.tensor, r0c - 1, [[EXT - 2, 128], [1, nr_h], [0, WP]]),
                    op=AOP.mult,
                )
                nsub += nr_h

            # conv2 chunk + residual + store
            o00, nrow_g2 = C2CH[gi]
            nsub = 0
            while nsub < nrow_g2:
                nr_h = min(4, nrow_g2 - nsub)
                o0 = o00 + nsub
                pc2 = ps.tile([C, 512], F32, tag="mm", name="pc2", space="PSUM")[:, : nr_h * W]
                for ki in range(3):
                    # conv2 out row o reads t1 rows (o+ki) in T1 coords
                    nc.tensor.matmul(
                        pc2[:, : nr_h * W],
                        w2psb[:, ki * C : (ki + 1) * C],
                        bass.AP(t13.tensor, (o0 + ki) * WP, [[T1S, 128], [WP, nr_h], [1, W]]),
                        start=(ki == 0), stop=False,
                    )
                    nc.tensor.matmul(
                        pc2[:, : nr_h * W],
                        w2ssb[:, ki * C : (ki + 1) * C],
                        bass.AP(t13.tensor, (o0 + ki) * WP + 2, [[T1S, C], [WP, nr_h], [1, W]]),
                        start=False, stop=(ki == 2),
                    )
                tf = gp.tile([C, 512], F32, tag="tf", name="tf", bufs=2)
                nc.scalar.activation(tf[:, : nr_h * W], pc2[:, :], ACTF.Identity, bias=b2sb[:, :])
                nc.vector.tensor_tensor(
                    out=tf[:, : nr_h * W].rearrange("p (r c) -> p r c", c=W),
                    in0=tf[:, : nr_h * W].rearrange("p (r c) -> p r c", c=W),
                    in1=bass.AP(os3.tensor, (o0 + 2) * WP + 1, [[EXT * WP, C], [WP, nr_h], [1, W]]),
                    op=AOP.add,
                )
                nc.sync.dma_start(
                    out=outp[:, o0 * W : (o0 + nr_h) * W], in_=tf[:, : nr_h * W]
                )
                nsub += nr_h

    nc.finalize()
    return nc


# ---------------- host side ----------------

_NC_CACHE = None


def _get_nc():
    global _NC_CACHE
    if _NC_CACHE is None:
        _NC_CACHE = build_nc()
    return _NC_CACHE


def _prep_core(inputs, b, q):
    G0 = 32 * q
    x = inputs["x"][b]  # [64, 128, 128]
    y = inputs["y"][b]

    def slab(img, lo, rows):
        out = np.zeros((C, rows, W), np.float32)
        for i in range(rows):
            g = lo + i
            if 0 <= g < 128:
                out[:, i, :] = img[:, g, :]
        return out

    import ml_dtypes
    bf = ml_dtypes.bfloat16
    xsl = slab(x, G0 - 3, XR).reshape(C, XR * W).astype(bf)
    ysl = slab(y, G0 - 12, YR).reshape(C, YR * W).astype(bf)

    w0 = inputs["w0"][:, :, 0, 0]                       # [o', e]
    w0d = np.concatenate([w0.T, w0.T], axis=1).copy().astype(bf)  # [e, 2*o]
    b0d = np.tile(inputs["b0"].reshape(C, 1), (2, 1)).astype(np.float32)

    # fold conv0 into the om conv's x half: om_x = (w_om_x @ w0) * x
    w_om = inputs["w_om"]                               # [27, 128, 3, 3]
    w_omx = np.einsum("ocij,ce->oeij", w_om[:, :C], w0).astype(np.float32)
    w_omc = np.concatenate([w_omx, w_om[:, C:]], axis=1)  # [27, 128, 3, 3]
    womt = (np.transpose(w_omc, (2, 3, 1, 0)).reshape(NK * 128, 27).copy()).astype(bf)

    wdct = (np.transpose(inputs["w_dc"], (2, 3, 1, 0)).reshape(NK * C, C).copy()).astype(bf)
    bdcd = np.tile(inputs["b_dc"].reshape(C, 1), (2, 1)).astype(np.float32)

    # conv1: M = 128 (dup); conv2: M = 64
    w1 = inputs["w1"]
    w1p = np.zeros((3, 128, 128), np.float32)
    w1s = np.zeros((3, C, 128), np.float32)
    for ki in range(3):
        lt0 = w1[:, :, ki, 0].T      # [c, o]
        lt1 = w1[:, :, ki, 1].T
        lt2 = w1[:, :, ki, 2].T
        w1p[ki, :C, :C] = lt0
        w1p[ki, :C, C:] = lt0
        w1p[ki, C:, :C] = lt1
        w1p[ki, C:, C:] = lt1
        w1s[ki, :, :C] = lt2
        w1s[ki, :, C:] = lt2
    b1dv = np.tile(inputs["b1"].reshape(C, 1), (2, 1)).astype(np.float32)

    w2 = inputs["w2"]
    w2p = np.zeros((3, 128, C), np.float32)
    w2s = np.zeros((3, C, C), np.float32)
    for ki in range(3):
        w2p[ki, :C, :] = w2[:, :, ki, 0].T
        w2p[ki, C:, :] = w2[:, :, ki, 1].T
        w2s[ki, :, :] = w2[:, :, ki, 2].T

    lo = 12.0 - G0
    hi = 139.0 - G0
    bnd = np.tile(np.array([[lo, hi, lo - 1.0, hi - 1.0]], np.float32), (128, 1))

    re_idx = np.arange(EXT)[:, None]
    ki = (np.arange(NK) // 3)[None, :]
    kj = (np.arange(NK) % 3)[None, :]
    crow_row = (re_idx + ki + 9).astype(np.float32).reshape(1, EXT * NK)
    crow = np.tile(crow_row, (128, 1))
    wv = np.arange(128)[:, None].astype(np.float32)
    cxw = (wv - 1.0 + kj.astype(np.float32))  # [128, 9]

    def rowmask(lo_r, rows):
        g = lo_r + np.arange(rows)
        m = ((g >= 0) & (g < 128)).astype(np.float32)
        return np.tile(m[None, :], (128, 1))

    return {
        "xs": xsl,
        "ys": ysl,
        "w0d": w0d,
        "b0d": b0d,
        "womt": womt,
        "bom": inputs["b_om"].reshape(27, 1).astype(np.float32),
        "wdct": wdct,
        "bdcd": bdcd,
        "w1p": w1p.reshape(3 * 128, 128).astype(bf),
        "w1s": w1s.reshape(3 * C, 128).astype(bf),
        "b1d": b1dv,
        "w2p": w2p.reshape(3 * 128, C).astype(bf),
        "w2s": w2s.reshape(3 * C, C).astype(bf),
        "b2": inputs["b2"].reshape(C, 1).astype(np.float32),
        "bnd": bnd,
        "crow": crow,
        "cxw": cxw.astype(np.float32),
        "m36d": rowmask(G0 - 2, EXT),
        "m34d": rowmask(G0 - 1, EXT - 2),
    }


def make_in_maps(inputs):
    inputs = {k: np.asarray(v, np.float32) for k, v in inputs.items()}
    return [_prep_core(inputs, core // 4, core % 4) for core in range(8)]


def kernel(**inputs):
    from concourse.bass_utils import run_bass_kernel_spmd

    nc = _get_nc()
    in_maps = make_in_maps(inputs)
    res = run_bass_kernel_spmd(nc, in_maps, core_ids=list(range(8)))
    out = np.zeros((2, C, 128, W), np.float32)
    for core in range(8):
        b, q = core // 4, core % 4
        out[b, :, 32 * q : 32 * q + 32, :] = res.results[core]["out"].reshape(C, OH, W)
    return out
